# revision 7
# baseline (speedup 1.0000x reference)
"""Trainium2 Bass kernel for nn_DiffSCM: diffusion MLP sampler.

Data-parallel over 8 NeuronCores (batch sharding, 32768 rows/core).
All activations live in transposed layout [D, rows]: the host pre-transposes
x0/noise/eps/x (and un-transposes the output), so the kernel needs ZERO
on-chip transposes. Per-row vectors (t, sqrt(1-t), 1-t, t_rand) broadcast
across partitions via K=1 ones-matmuls into PSUM. Matmuls run in float32r
(fast fp32, ~1e-4 rel err). Softplus = ln(1+e^x) via ACT Exp + bit-hack log
seed + one Newton step (single ACT table, no table loads). The mu-head
epilogue (mu + bm - x + sig*eps) is accumulated INSIDE the mm3 PSUM using
bias-row, -I and +I matmuls. Elementwise is balanced across DVE and GPSIMD.

reference:
  t = linspace(0,1,B)[:,None]
  xt' = x0 + noise*sqrt(1-t)
  h   = relu([xt', t] @ W1.T + b1)
  ft  = tanh(h @ W2.T + b2)
  xt  = xt' + (1-t)*ft
  mu  = xt @ Wm.T + bm
  sig = softplus(xt @ Wv.T + bv)
  out = (1-t_rand)*x + t_rand*(mu + sig*eps)
"""
import numpy as np

import concourse.bass as bass
import concourse.bacc as bacc
import concourse.tile as tile
from concourse import mybir
from concourse.bass_utils import run_bass_kernel_spmd

F32 = mybir.dt.float32
F32R = mybir.dt.float32r
I32 = mybir.dt.int32
AF = mybir.ActivationFunctionType
OP = mybir.AluOpType

B, D = 262144, 256
NCORES = 8
RS = B // NCORES          # rows per core shard = 32768
R = 512                   # rows per block
NBLK = RS // R            # 64 blocks
OC = D // 128             # feature chunks = 2

# softplus bit-hack constants: with y0m = float(bits(z) - BP),
# ln(z) ~ C*y0m + 1  (the -1/C offset is folded into BP)
BP = 1077091419
C = float(np.log(2.0) / 2**23)

_nc_cache = {}


def _bc(ap, n):
    """[128, F] AP -> [128, n, F] AP with broadcast (step-0) middle dim."""
    return bass.AP(tensor=ap.tensor, offset=ap.offset,
                   ap=[list(ap.ap[0]), [0, n], list(ap.ap[1])])


def build_nc():
    if "nc" in _nc_cache:
        return _nc_cache["nc"]
    nc = bacc.Bacc("TRN2")

    X0T = nc.dram_tensor("x0t", [OC, 128, RS], F32, kind="ExternalInput")
    NST = nc.dram_tensor("nst", [OC, 128, RS], F32, kind="ExternalInput")
    EPT = nc.dram_tensor("ept", [OC, 128, RS], F32, kind="ExternalInput")
    XT = nc.dram_tensor("xt", [OC, 128, RS], F32, kind="ExternalInput")
    # row vectors: [1, vec(t, s1mt, omt, trand), RS]
    VT = nc.dram_tensor("vt", [1, 4, RS], F32, kind="ExternalInput")
    # weights pre-transposed: [kc, p, m] with W?T[kc, p, m] = W[m, kc*128+p]
    W1AT = nc.dram_tensor("w1at", [OC, 128, D], F32, kind="ExternalInput")
    W2T = nc.dram_tensor("w2t", [OC, 128, D], F32, kind="ExternalInput")
    WMT = nc.dram_tensor("wmt", [OC, 128, D], F32, kind="ExternalInput")
    WVT = nc.dram_tensor("wvt", [OC, 128, D], F32, kind="ExternalInput")
    W1L = nc.dram_tensor("w1l", [1, D], F32, kind="ExternalInput")
    BMR = nc.dram_tensor("bmr", [1, D], F32, kind="ExternalInput")
    B1T = nc.dram_tensor("b1t", [128, OC], F32, kind="ExternalInput")
    B2T = nc.dram_tensor("b2t", [128, OC], F32, kind="ExternalInput")
    BVT = nc.dram_tensor("bvt", [128, OC], F32, kind="ExternalInput")
    IDT = nc.dram_tensor("idt", [128, 128], F32, kind="ExternalInput")
    NEGI = nc.dram_tensor("negi", [128, 128], F32, kind="ExternalInput")
    ONES = nc.dram_tensor("ones1", [1, 128], F32, kind="ExternalInput")
    ONESR = nc.dram_tensor("onesr", [1, R], F32, kind="ExternalInput")
    OUTT = nc.dram_tensor("outt", [OC, 128, RS], F32, kind="ExternalOutput")

    x0_v = X0T.rearrange("c p f -> p c f")
    ns_v = NST.rearrange("c p f -> p c f")
    ep_v = EPT.rearrange("c p f -> p c f")
    x_v = XT.rearrange("c p f -> p c f")
    out_v = OUTT.rearrange("c p f -> p c f")

    with tile.TileContext(nc) as tc:
        with tc.tile_pool(name="consts", bufs=1) as cp, \
             tc.tile_pool(name="loads", bufs=3) as lp, \
             tc.tile_pool(name="work", bufs=2) as wp, \
             tc.tile_pool(name="psum", bufs=2, space="PSUM") as pp:

            # ---- one-time constants ----
            w1at = cp.tile([128, OC, D], F32R)
            w2t = cp.tile([128, OC, D], F32R)
            wmt = cp.tile([128, OC, D], F32R)
            wvt = cp.tile([128, OC, D], F32R)
            nc.sync.dma_start(out=w1at, in_=W1AT.rearrange("k p m -> p k m").bitcast(F32R))
            nc.sync.dma_start(out=w2t, in_=W2T.rearrange("k p m -> p k m").bitcast(F32R))
            nc.sync.dma_start(out=wmt, in_=WMT.rearrange("k p m -> p k m").bitcast(F32R))
            nc.sync.dma_start(out=wvt, in_=WVT.rearrange("k p m -> p k m").bitcast(F32R))
            w1l = cp.tile([1, D], F32R)
            bmr = cp.tile([1, D], F32R)
            idt = cp.tile([128, 128], F32R)
            negi = cp.tile([128, 128], F32R)
            ones = cp.tile([1, 128], F32R)
            onesr = cp.tile([1, R], F32R)
            nc.sync.dma_start(out=w1l, in_=W1L[:, :].bitcast(F32R))
            nc.sync.dma_start(out=bmr, in_=BMR[:, :].bitcast(F32R))
            nc.sync.dma_start(out=idt, in_=IDT[:, :].bitcast(F32R))
            nc.sync.dma_start(out=negi, in_=NEGI[:, :].bitcast(F32R))
            nc.sync.dma_start(out=ones, in_=ONES[:, :].bitcast(F32R))
            nc.sync.dma_start(out=onesr, in_=ONESR[:, :].bitcast(F32R))
            b1t = cp.tile([128, OC], F32)
            b2t = cp.tile([128, OC], F32)
            bvt = cp.tile([128, OC], F32)
            nc.sync.dma_start(out=b1t, in_=B1T[:, :])
            nc.sync.dma_start(out=b2t, in_=B2T[:, :])
            nc.sync.dma_start(out=bvt, in_=BVT[:, :])
            neg1 = cp.tile([128, 1], F32)
            nc.vector.memset(neg1, -1.0)

            for b in range(NBLK):
                cols = slice(b * R, (b + 1) * R)

                x0t = lp.tile([128, OC, R], F32, tag="x0t")
                nst = lp.tile([128, OC, R], F32, tag="nst")
                ept = lp.tile([128, OC, R], F32, tag="ept")
                xt = lp.tile([128, OC, R], F32R, tag="xt")
                vt = lp.tile([1, 4, R], F32R, tag="vt")
                nc.sync.dma_start(out=x0t, in_=x0_v[:, :, cols])
                nc.sync.dma_start(out=nst, in_=ns_v[:, :, cols])
                nc.sync.dma_start(out=ept, in_=ep_v[:, :, cols])
                nc.sync.dma_start(out=xt, in_=x_v[:, :, cols].bitcast(F32R))
                nc.sync.dma_start(out=vt, in_=VT[:, :, cols].bitcast(F32R))

                # broadcasts: psum[p, r] = vec[r]
                s1b = pp.tile([128, R], F32, tag="bc")
                omb = pp.tile([128, R], F32, tag="bc")
                trb = pp.tile([128, R], F32, tag="bc")
                nc.tensor.matmul(s1b, ones, vt[:, 1, :], start=True, stop=True)
                nc.tensor.matmul(omb, ones, vt[:, 2, :], start=True, stop=True)
                nc.tensor.matmul(trb, ones, vt[:, 3, :], start=True, stop=True)
                trs = wp.tile([128, R], F32, tag="trs")
                nc.scalar.copy(out=trs, in_=trb)

                # prologue: xt' = x0 + noise*sqrt(1-t)
                xtp = wp.tile([128, OC, R], F32R, tag="xtp")
                nc.vector.tensor_tensor(xtp, nst, _bc(s1b, OC), OP.mult)   # DVE
                nc.vector.tensor_tensor(xtp, xtp, x0t, OP.add)             # DVE

                # layer 1: h = relu(W1a @ xt'T + w1last x t + b1)
                ph = pp.tile([128, OC, R], F32, tag="mm")
                ht = wp.tile([128, OC, R], F32R, tag="ht")
                for oc in range(OC):
                    ocs = slice(oc * 128, (oc + 1) * 128)
                    nc.tensor.matmul(ph[:, oc, :], w1at[:, 0, ocs], xtp[:, 0, :], start=True, stop=False)
                    nc.tensor.matmul(ph[:, oc, :], w1at[:, 1, ocs], xtp[:, 1, :], start=False, stop=False)
                    nc.tensor.matmul(ph[:, oc, :], w1l[:, ocs], vt[:, 0, :], start=False, stop=True)
                    nc.scalar.activation(out=ht[:, oc, :], in_=ph[:, oc, :],
                                         func=AF.Relu, bias=b1t[:, oc:oc + 1])

                # layer 2: ft = tanh(W2 @ h + b2)
                pf = pp.tile([128, OC, R], F32, tag="mm")
                ftt = wp.tile([128, OC, R], F32, tag="ftt")
                for oc in range(OC):
                    ocs = slice(oc * 128, (oc + 1) * 128)
                    nc.tensor.matmul(pf[:, oc, :], w2t[:, 0, ocs], ht[:, 0, :], start=True, stop=False)
                    nc.tensor.matmul(pf[:, oc, :], w2t[:, 1, ocs], ht[:, 1, :], start=False, stop=True)
                    nc.scalar.activation(out=ftt[:, oc, :], in_=pf[:, oc, :],
                                         func=AF.Tanh, bias=b2t[:, oc:oc + 1])

                # xtT = xt'T + (1-t)*ftT
                xtt = wp.tile([128, OC, R], F32R, tag="xtt")
                nc.vector.tensor_tensor(xtt, ftt, _bc(omb, OC), OP.mult)   # DVE
                nc.gpsimd.tensor_tensor(xtt, xtt, xtp, OP.add)             # GPS

                # sigma head: xv = Wv @ xtT + bv ; sig = softplus(xv)
                pv = pp.tile([128, OC, R], F32, tag="mm")
                w = wp.tile([128, OC, R], F32, tag="w")
                for oc in range(OC):
                    ocs = slice(oc * 128, (oc + 1) * 128)
                    nc.tensor.matmul(pv[:, oc, :], wvt[:, 0, ocs], xtt[:, 0, :], start=True, stop=False)
                    nc.tensor.matmul(pv[:, oc, :], wvt[:, 1, ocs], xtt[:, 1, :], start=False, stop=True)
                    # w = exp(xv)
                    nc.scalar.activation(out=w[:, oc, :], in_=pv[:, oc, :],
                                         func=AF.Exp, bias=bvt[:, oc:oc + 1])
                # z = w + 1 (ACT), bit-hack seed, e = exp(-C*y0m - 1) (ACT)
                z = wp.tile([128, OC, R], F32, tag="z")
                nc.scalar.add(z, w, 1.0)
                y0m = wp.tile([128, OC, R], F32, tag="y0m")
                nc.vector.tensor_scalar(y0m, z.bitcast(I32), BP, None, OP.subtract)  # DVE
                e = wp.tile([128, OC, R], F32, tag="e")
                nc.scalar.activation(out=e, in_=y0m, func=AF.Exp, bias=neg1[:, :], scale=-C)
                nc.gpsimd.tensor_tensor(z, z, e, OP.mult)                  # GPS: z := u
                sg = wp.tile([128, OC, R], F32R, tag="sg")
                nc.vector.scalar_tensor_tensor(sg, y0m, C, z, OP.mult, OP.add)  # DVE: sigma
                # sg := sigma * eps
                nc.vector.tensor_tensor(sg, sg, ept, OP.mult)              # DVE

                # mu head, fully accumulated in PSUM:
                # pm = Wm @ xtT + bm x 1 - x + sig*eps   (= x' - x)
                pm = pp.tile([128, OC, R], F32, tag="pm", bufs=1)
                for oc in range(OC):
                    ocs = slice(oc * 128, (oc + 1) * 128)
                    nc.tensor.matmul(pm[:, oc, :], wmt[:, 0, ocs], xtt[:, 0, :], start=True, stop=False)
                    nc.tensor.matmul(pm[:, oc, :], wmt[:, 1, ocs], xtt[:, 1, :], start=False, stop=False)
                    nc.tensor.matmul(pm[:, oc, :], bmr[:, ocs], onesr, start=False, stop=False)
                    nc.tensor.matmul(pm[:, oc, :], negi, xt[:, oc, :], start=False, stop=False)
                    nc.tensor.matmul(pm[:, oc, :], idt, sg[:, oc, :], start=False, stop=True)

                # out = x + t_rand * (x' - x)
                ep = wp.tile([128, OC, R], F32, tag="ep")
                nc.vector.tensor_tensor(ep, pm, _bc(trs, OC), OP.mult)     # DVE
                outt = wp.tile([128, OC, R], F32, tag="outt")
                nc.gpsimd.tensor_tensor(outt, ep, xt.bitcast(F32), OP.add)  # GPS

                nc.sync.dma_start(out=out_v[:, :, cols], in_=outt)

    nc.finalize()
    _nc_cache["nc"] = nc
    return nc


def _prep_inputs(x, noise, x0, t_rand, eps, W1, b1, W2, b2, Wm, bm, Wv, bv):
    """Shard + transpose on host; returns in_maps for the 8 cores."""
    t = np.linspace(0.0, 1.0, B, dtype=np.float32)
    s1mt = np.sqrt(1.0 - t, dtype=np.float32)
    omt = (1.0 - t).astype(np.float32)
    tr = np.ascontiguousarray(t_rand[:, 0])

    def wT(W):  # [D, D] -> [OC, 128, D] with out[k, p, m] = W[m, k*128+p]
        return np.ascontiguousarray(W.T.reshape(OC, 128, D))

    def bT(b):  # [D] -> [128, OC]
        return np.ascontiguousarray(b.reshape(OC, 128).T)

    shared = {
        "w1at": wT(np.ascontiguousarray(W1[:, :D])),
        "w2t": wT(W2), "wmt": wT(Wm), "wvt": wT(Wv),
        "w1l": np.ascontiguousarray(W1[:, D]).reshape(1, D),
        "bmr": bm.reshape(1, D),
        "b1t": bT(b1), "b2t": bT(b2), "bvt": bT(bv),
        "idt": np.eye(128, dtype=np.float32),
        "negi": (-np.eye(128)).astype(np.float32),
        "ones1": np.ones((1, 128), dtype=np.float32),
        "onesr": np.ones((1, R), dtype=np.float32),
    }

    def shardT(M, c):  # [B, D] -> [OC, 128, RS] transposed shard
        sh = M[c * RS:(c + 1) * RS, :]
        return np.ascontiguousarray(sh.T.reshape(OC, 128, RS))

    in_maps = []
    for c in range(NCORES):
        sl = slice(c * RS, (c + 1) * RS)
        vtv = np.stack([t[sl], s1mt[sl], omt[sl], tr[sl]], axis=0)[None]
        in_maps.append({
            "x0t": shardT(x0, c), "nst": shardT(noise, c),
            "ept": shardT(eps, c), "xt": shardT(x, c),
            "vt": np.ascontiguousarray(vtv), **shared,
        })
    return in_maps


def _run(in_maps, trace=False):
    nc = build_nc()
    return run_bass_kernel_spmd(nc, in_maps, list(range(NCORES)), trace=trace)


def _assemble(results):
    out = np.empty((B, D), dtype=np.float32)
    for c in range(NCORES):
        o = results[c]["outt"].reshape(D, RS)
        out[c * RS:(c + 1) * RS, :] = o.T
    return out


def kernel(**inputs) -> np.ndarray:
    in_maps = _prep_inputs(**inputs)
    res = _run(in_maps, trace=False)
    return _assemble(res.results)


def kernel_traced(**inputs):
    """Same as kernel() but with NTFF tracing; returns (out, exec_time_ns, results)."""
    in_maps = _prep_inputs(**inputs)
    res = _run(in_maps, trace=True)
    return _assemble(res.results), res.exec_time_ns, res


# revision 8
# speedup vs baseline: 1.0518x; 1.0518x over previous
"""Trainium2 Bass kernel for nn_DiffSCM: diffusion MLP sampler.

Data-parallel over 8 NeuronCores (batch sharding, 32768 rows/core).
All activations live in transposed layout [D, rows]: the host pre-transposes
x0/noise/eps/x (and un-transposes the output), so the kernel needs ZERO
on-chip transposes. Per-row vectors (t, sqrt(1-t), 1-t, t_rand) broadcast
across partitions via K=1 ones-matmuls into PSUM. Matmuls run in float32r
(fast fp32, ~1e-4 rel err). Softplus = ln(1+e^x) via ACT Exp + bit-hack log
seed + one Newton step (single ACT table, no table loads). The mu-head
epilogue (mu + bm - x + sig*eps) is accumulated INSIDE the mm3 PSUM using
bias-row, -I and +I matmuls. Elementwise is balanced across DVE and GPSIMD.

reference:
  t = linspace(0,1,B)[:,None]
  xt' = x0 + noise*sqrt(1-t)
  h   = relu([xt', t] @ W1.T + b1)
  ft  = tanh(h @ W2.T + b2)
  xt  = xt' + (1-t)*ft
  mu  = xt @ Wm.T + bm
  sig = softplus(xt @ Wv.T + bv)
  out = (1-t_rand)*x + t_rand*(mu + sig*eps)
"""
import numpy as np

import concourse.bass as bass
import concourse.bacc as bacc
import concourse.tile as tile
from concourse import mybir
from concourse.bass_utils import run_bass_kernel_spmd

F32 = mybir.dt.float32
F32R = mybir.dt.float32r
I32 = mybir.dt.int32
AF = mybir.ActivationFunctionType
OP = mybir.AluOpType

B, D = 262144, 256
NCORES = 8
RS = B // NCORES          # rows per core shard = 32768
R = 512                   # rows per block
NBLK = RS // R            # 64 blocks
OC = D // 128             # feature chunks = 2

# softplus bit-hack constants: with y0m = float(bits(z) - BP),
# ln(z) ~ C*y0m + 1  (the -1/C offset is folded into BP)
BP = 1077091419
C = float(np.log(2.0) / 2**23)

_nc_cache = {}


def _bc(ap, n):
    """[128, F] AP -> [128, n, F] AP with broadcast (step-0) middle dim."""
    return bass.AP(tensor=ap.tensor, offset=ap.offset,
                   ap=[list(ap.ap[0]), [0, n], list(ap.ap[1])])


def build_nc():
    if "nc" in _nc_cache:
        return _nc_cache["nc"]
    nc = bacc.Bacc("TRN2")

    X0T = nc.dram_tensor("x0t", [OC, 128, RS], F32, kind="ExternalInput")
    NST = nc.dram_tensor("nst", [OC, 128, RS], F32, kind="ExternalInput")
    EPT = nc.dram_tensor("ept", [OC, 128, RS], F32, kind="ExternalInput")
    XT = nc.dram_tensor("xt", [OC, 128, RS], F32, kind="ExternalInput")
    # row vectors: [1, vec(t, s1mt, omt, trand), RS]
    VT = nc.dram_tensor("vt", [1, 4, RS], F32, kind="ExternalInput")
    # weights pre-transposed: [kc, p, m] with W?T[kc, p, m] = W[m, kc*128+p]
    W1AT = nc.dram_tensor("w1at", [OC, 128, D], F32, kind="ExternalInput")
    W2T = nc.dram_tensor("w2t", [OC, 128, D], F32, kind="ExternalInput")
    WMT = nc.dram_tensor("wmt", [OC, 128, D], F32, kind="ExternalInput")
    WVT = nc.dram_tensor("wvt", [OC, 128, D], F32, kind="ExternalInput")
    W1L = nc.dram_tensor("w1l", [1, D], F32, kind="ExternalInput")
    BMR = nc.dram_tensor("bmr", [1, D], F32, kind="ExternalInput")
    B1T = nc.dram_tensor("b1t", [128, OC], F32, kind="ExternalInput")
    B2T = nc.dram_tensor("b2t", [128, OC], F32, kind="ExternalInput")
    BVT = nc.dram_tensor("bvt", [128, OC], F32, kind="ExternalInput")
    IDT = nc.dram_tensor("idt", [128, 128], F32, kind="ExternalInput")
    NEGI = nc.dram_tensor("negi", [128, 128], F32, kind="ExternalInput")
    ONES = nc.dram_tensor("ones1", [1, 128], F32, kind="ExternalInput")
    ONESR = nc.dram_tensor("onesr", [1, R], F32, kind="ExternalInput")
    OUTT = nc.dram_tensor("outt", [OC, 128, RS], F32, kind="ExternalOutput")

    x0_v = X0T.rearrange("c p f -> p c f")
    ns_v = NST.rearrange("c p f -> p c f")
    ep_v = EPT.rearrange("c p f -> p c f")
    x_v = XT.rearrange("c p f -> p c f")
    out_v = OUTT.rearrange("c p f -> p c f")

    with tile.TileContext(nc) as tc:
        with tc.tile_pool(name="consts", bufs=1) as cp, \
             tc.tile_pool(name="loads", bufs=3) as lp, \
             tc.tile_pool(name="work", bufs=2) as wp, \
             tc.tile_pool(name="psum", bufs=2, space="PSUM") as pp:

            # ---- one-time constants ----
            w1at = cp.tile([128, OC, D], F32R)
            w2t = cp.tile([128, OC, D], F32R)
            wmt = cp.tile([128, OC, D], F32R)
            wvt = cp.tile([128, OC, D], F32R)
            nc.sync.dma_start(out=w1at, in_=W1AT.rearrange("k p m -> p k m").bitcast(F32R))
            nc.sync.dma_start(out=w2t, in_=W2T.rearrange("k p m -> p k m").bitcast(F32R))
            nc.sync.dma_start(out=wmt, in_=WMT.rearrange("k p m -> p k m").bitcast(F32R))
            nc.sync.dma_start(out=wvt, in_=WVT.rearrange("k p m -> p k m").bitcast(F32R))
            w1l = cp.tile([1, D], F32R)
            bmr = cp.tile([1, D], F32R)
            idt = cp.tile([128, 128], F32R)
            negi = cp.tile([128, 128], F32R)
            ones = cp.tile([1, 128], F32R)
            onesr = cp.tile([1, R], F32R)
            nc.sync.dma_start(out=w1l, in_=W1L[:, :].bitcast(F32R))
            nc.sync.dma_start(out=bmr, in_=BMR[:, :].bitcast(F32R))
            nc.sync.dma_start(out=idt, in_=IDT[:, :].bitcast(F32R))
            nc.sync.dma_start(out=negi, in_=NEGI[:, :].bitcast(F32R))
            nc.sync.dma_start(out=ones, in_=ONES[:, :].bitcast(F32R))
            nc.sync.dma_start(out=onesr, in_=ONESR[:, :].bitcast(F32R))
            b1t = cp.tile([128, OC], F32)
            b2t = cp.tile([128, OC], F32)
            bvt = cp.tile([128, OC], F32)
            nc.sync.dma_start(out=b1t, in_=B1T[:, :])
            nc.sync.dma_start(out=b2t, in_=B2T[:, :])
            nc.sync.dma_start(out=bvt, in_=BVT[:, :])
            neg1 = cp.tile([128, 1], F32)
            nc.vector.memset(neg1, -1.0)

            for b in range(NBLK):
                cols = slice(b * R, (b + 1) * R)

                x0t = lp.tile([128, OC, R], F32, tag="x0t")
                nst = lp.tile([128, OC, R], F32, tag="nst")
                ept = lp.tile([128, OC, R], F32, tag="ept")
                xt = lp.tile([128, OC, R], F32R, tag="xt")
                vt = lp.tile([1, 4, R], F32R, tag="vt")
                nc.sync.dma_start(out=x0t, in_=x0_v[:, :, cols])
                nc.sync.dma_start(out=nst, in_=ns_v[:, :, cols])
                nc.sync.dma_start(out=ept, in_=ep_v[:, :, cols])
                nc.sync.dma_start(out=xt, in_=x_v[:, :, cols].bitcast(F32R))
                nc.sync.dma_start(out=vt, in_=VT[:, :, cols].bitcast(F32R))

                # broadcasts: psum[p, r] = vec[r]
                s1b = pp.tile([128, R], F32, tag="bc", bufs=1)
                omb = pp.tile([128, R], F32, tag="bc", bufs=1)
                trb = pp.tile([128, R], F32, tag="bc", bufs=1)
                nc.tensor.matmul(s1b, ones, vt[:, 1, :], start=True, stop=True)
                nc.tensor.matmul(omb, ones, vt[:, 2, :], start=True, stop=True)
                nc.tensor.matmul(trb, ones, vt[:, 3, :], start=True, stop=True)
                trs = wp.tile([128, R], F32, tag="trs")
                nc.scalar.copy(out=trs, in_=trb)

                # prologue: xt' = x0 + noise*sqrt(1-t)
                xtp = wp.tile([128, OC, R], F32R, tag="xtp")
                nc.vector.tensor_tensor(xtp, nst, _bc(s1b, OC), OP.mult)   # DVE
                nc.vector.tensor_tensor(xtp, xtp, x0t, OP.add)             # DVE

                # layer 1: h = relu(W1a @ xt'T + w1last x t + b1)
                ph = pp.tile([128, OC, R], F32, tag="mm", bufs=3)
                ht = wp.tile([128, OC, R], F32R, tag="ht")
                for oc in range(OC):
                    ocs = slice(oc * 128, (oc + 1) * 128)
                    nc.tensor.matmul(ph[:, oc, :], w1at[:, 0, ocs], xtp[:, 0, :], start=True, stop=False)
                    nc.tensor.matmul(ph[:, oc, :], w1at[:, 1, ocs], xtp[:, 1, :], start=False, stop=False)
                    nc.tensor.matmul(ph[:, oc, :], w1l[:, ocs], vt[:, 0, :], start=False, stop=True)
                    nc.scalar.activation(out=ht[:, oc, :], in_=ph[:, oc, :],
                                         func=AF.Relu, bias=b1t[:, oc:oc + 1])

                # layer 2: ft = tanh(W2 @ h + b2)
                pf = pp.tile([128, OC, R], F32, tag="mm", bufs=3)
                ftt = wp.tile([128, OC, R], F32, tag="ftt")
                for oc in range(OC):
                    ocs = slice(oc * 128, (oc + 1) * 128)
                    nc.tensor.matmul(pf[:, oc, :], w2t[:, 0, ocs], ht[:, 0, :], start=True, stop=False)
                    nc.tensor.matmul(pf[:, oc, :], w2t[:, 1, ocs], ht[:, 1, :], start=False, stop=True)
                    nc.scalar.activation(out=ftt[:, oc, :], in_=pf[:, oc, :],
                                         func=AF.Tanh, bias=b2t[:, oc:oc + 1])

                # xtT = xt'T + (1-t)*ftT
                xtt = wp.tile([128, OC, R], F32R, tag="xtt")
                nc.vector.tensor_tensor(xtt, ftt, _bc(omb, OC), OP.mult)   # DVE
                nc.gpsimd.tensor_tensor(xtt, xtt, xtp, OP.add)             # GPS

                # sigma head: xv = Wv @ xtT + bv ; sig = softplus(xv)
                pv = pp.tile([128, OC, R], F32, tag="mm", bufs=3)
                w = wp.tile([128, OC, R], F32, tag="w")
                for oc in range(OC):
                    ocs = slice(oc * 128, (oc + 1) * 128)
                    nc.tensor.matmul(pv[:, oc, :], wvt[:, 0, ocs], xtt[:, 0, :], start=True, stop=False)
                    nc.tensor.matmul(pv[:, oc, :], wvt[:, 1, ocs], xtt[:, 1, :], start=False, stop=True)
                    # w = exp(xv)
                    nc.scalar.activation(out=w[:, oc, :], in_=pv[:, oc, :],
                                         func=AF.Exp, bias=bvt[:, oc:oc + 1])
                # z = w + 1 (ACT), bit-hack seed, e = exp(-C*y0m - 1) (ACT)
                z = wp.tile([128, OC, R], F32, tag="z")
                nc.scalar.add(z, w, 1.0)
                y0m = wp.tile([128, OC, R], F32, tag="y0m")
                nc.vector.tensor_scalar(y0m, z.bitcast(I32), BP, None, OP.subtract)  # DVE
                e = wp.tile([128, OC, R], F32, tag="e")
                nc.scalar.activation(out=e, in_=y0m, func=AF.Exp, bias=neg1[:, :], scale=-C)
                nc.gpsimd.tensor_tensor(z, z, e, OP.mult)                  # GPS: z := u
                sg = wp.tile([128, OC, R], F32, tag="sg")
                nc.vector.scalar_tensor_tensor(sg, y0m, C, z, OP.mult, OP.add)  # DVE: sigma
                sge = wp.tile([128, OC, R], F32R, tag="sge")
                nc.vector.tensor_tensor(sge, sg, ept, OP.mult)             # DVE: sigma*eps

                # mu head, fully accumulated in PSUM:
                # pm = Wm @ xtT + bm x 1 - x + sig*eps   (= x' - x)
                pm = pp.tile([128, OC, R], F32, tag="mm", bufs=3)
                for oc in range(OC):
                    ocs = slice(oc * 128, (oc + 1) * 128)
                    nc.tensor.matmul(pm[:, oc, :], wmt[:, 0, ocs], xtt[:, 0, :], start=True, stop=False)
                    nc.tensor.matmul(pm[:, oc, :], wmt[:, 1, ocs], xtt[:, 1, :], start=False, stop=False)
                    nc.tensor.matmul(pm[:, oc, :], bmr[:, ocs], onesr, start=False, stop=False)
                    nc.tensor.matmul(pm[:, oc, :], negi, xt[:, oc, :], start=False, stop=False)
                    nc.tensor.matmul(pm[:, oc, :], idt, sge[:, oc, :], start=False, stop=True)

                # out = x + t_rand * (x' - x)
                ep = wp.tile([128, OC, R], F32, tag="ep")
                nc.vector.tensor_tensor(ep, pm, _bc(trs, OC), OP.mult)     # DVE
                outt = wp.tile([128, OC, R], F32, tag="outt")
                nc.gpsimd.tensor_tensor(outt, ep, xt.bitcast(F32), OP.add)  # GPS

                nc.sync.dma_start(out=out_v[:, :, cols], in_=outt)

    nc.finalize()
    _nc_cache["nc"] = nc
    return nc


def _prep_inputs(x, noise, x0, t_rand, eps, W1, b1, W2, b2, Wm, bm, Wv, bv):
    """Shard + transpose on host; returns in_maps for the 8 cores."""
    t = np.linspace(0.0, 1.0, B, dtype=np.float32)
    s1mt = np.sqrt(1.0 - t, dtype=np.float32)
    omt = (1.0 - t).astype(np.float32)
    tr = np.ascontiguousarray(t_rand[:, 0])

    def wT(W):  # [D, D] -> [OC, 128, D] with out[k, p, m] = W[m, k*128+p]
        return np.ascontiguousarray(W.T.reshape(OC, 128, D))

    def bT(b):  # [D] -> [128, OC]
        return np.ascontiguousarray(b.reshape(OC, 128).T)

    shared = {
        "w1at": wT(np.ascontiguousarray(W1[:, :D])),
        "w2t": wT(W2), "wmt": wT(Wm), "wvt": wT(Wv),
        "w1l": np.ascontiguousarray(W1[:, D]).reshape(1, D),
        "bmr": bm.reshape(1, D),
        "b1t": bT(b1), "b2t": bT(b2), "bvt": bT(bv),
        "idt": np.eye(128, dtype=np.float32),
        "negi": (-np.eye(128)).astype(np.float32),
        "ones1": np.ones((1, 128), dtype=np.float32),
        "onesr": np.ones((1, R), dtype=np.float32),
    }

    def shardT(M, c):  # [B, D] -> [OC, 128, RS] transposed shard
        sh = M[c * RS:(c + 1) * RS, :]
        return np.ascontiguousarray(sh.T.reshape(OC, 128, RS))

    in_maps = []
    for c in range(NCORES):
        sl = slice(c * RS, (c + 1) * RS)
        vtv = np.stack([t[sl], s1mt[sl], omt[sl], tr[sl]], axis=0)[None]
        in_maps.append({
            "x0t": shardT(x0, c), "nst": shardT(noise, c),
            "ept": shardT(eps, c), "xt": shardT(x, c),
            "vt": np.ascontiguousarray(vtv), **shared,
        })
    return in_maps


def _run(in_maps, trace=False):
    nc = build_nc()
    return run_bass_kernel_spmd(nc, in_maps, list(range(NCORES)), trace=trace)


def _assemble(results):
    out = np.empty((B, D), dtype=np.float32)
    for c in range(NCORES):
        o = results[c]["outt"].reshape(D, RS)
        out[c * RS:(c + 1) * RS, :] = o.T
    return out


def kernel(**inputs) -> np.ndarray:
    in_maps = _prep_inputs(**inputs)
    res = _run(in_maps, trace=False)
    return _assemble(res.results)


def kernel_traced(**inputs):
    """Same as kernel() but with NTFF tracing; returns (out, exec_time_ns, results)."""
    in_maps = _prep_inputs(**inputs)
    res = _run(in_maps, trace=True)
    return _assemble(res.results), res.exec_time_ns, res
